# revision 20
# baseline (speedup 1.0000x reference)
"""ROI-Align + MLP classification head (nms_detection) on 8 Trainium2 cores.

Strategy: data-parallel over batch (2 images per core). Host-side prep keeps
only layout/dtype work off-device: the feature map is cast to fp16 and stored
y-pair-interleaved (fm2[b,y,x] = [fm[y,x], fm[y+1,x]]) so ONE 2KB indirect-DMA
descriptor fetches all 4 bilinear corners of a sample; weights are pre-cast to
fp16 in their final SBUF layouts; proposals are pre-expanded to the
per-partition slot layout with the image offset folded into the y coords.

Per core: 44 rois x 16 bin centers = 704 samples in 6 groups of 128.
  sample partition p = A*16 + q (A = p//16, q = iy*4+ix), roi slot = A*6 + g.
Device: short index chain (split vector/gpsimd) -> 6 indirect gathers
(one per group, 128 x 2KB descriptors) -> per-group bilinear combine (vector)
+ PE transpose -> 3-layer MLP (fp16 in / fp32 psum) -> fp32 softmax.
"""

import numpy as np

import concourse.bacc as bacc
import concourse.bass as bass
import concourse.mybir as mybir
import concourse.tile as tile
from concourse._compat import get_trn_type
from concourse.bass_utils import run_bass_kernel_spmd

# Problem shape (hardcoded per contract)
B, P, H, W, C = 16, 22, 128, 128, 256
NUM_CLASSES = 10
N_CORES = 8
B_LOC = B // N_CORES        # 2 images per core
NROI = B_LOC * P            # 44 rois per core
NRS = 48                    # roi slots (8 partition-blocks x 6 groups)
NG = 6                      # sample groups of 128
HID1, HID2 = 128, 64
F32 = mybir.dt.float32
F16 = mybir.dt.float16
I32 = mybir.dt.int32
AX_X = mybir.AxisListType.X
OP = mybir.AluOpType
AF = mybir.ActivationFunctionType

NPIX = B_LOC * H * W            # 32768 flat (b, y, x) rows in fm2
MAGIC = 12582912.0              # 1.5 * 2^23 fp32 round-to-int magic
NCB = 38                        # f32 const bundle cols
NFB = 128 + HID2 + NUM_CLASSES  # f16 const bundle cols (ident | W2 | W3)


def _emit_body(nc, tc, fm2, cb, fb, w1p, out, cpool, wpool, ppool):
    V = nc.vector
    GP = nc.gpsimd

    # ---------------- const loads (W1 load deferred, see below) ----------
    CB = cpool.tile([128, NCB], F32, name="CB")
    nc.sync.dma_start(CB[:], cb)
    FB = cpool.tile([128, NFB], F16, name="FB")
    nc.sync.dma_start(FB[:], fb)
    W1sb = cpool.tile([128, 4096], F16, name="W1sb")

    ident = FB[:, 0:128]
    W2sb = FB[:, 128:128 + HID2]
    W3sb = FB[0:HID2 + 1, 128 + HID2:NFB]   # rows 0-63 = W3, row 64 = b3
    b1sb = CB[:, 26:27]
    b2sb = CB[0:HID2, 27:28]
    cgv = CB[:, 0:24].rearrange("p (g k) -> p g k", g=NG)
    y1c, x1c, dyc, dxc = (cgv[:, :, k] for k in range(4))
    cy_ap, cx_ap = CB[:, 24:25], CB[:, 25:26]

    # ---------------- index chain (critical path to the gathers) --------
    # Split into group halves so the first 3 gathers issue while the second
    # half of the chain still runs. x chain on gpsimd, y chain on vector.
    def t6(name):
        return wpool.tile([128, NG], F32, name=name)

    ys, y0f = t6("ys"), t6("y0f")
    xs, x0f = t6("xs"), t6("x0f")
    pixf = t6("pixf")
    idx = cpool.tile([128, NG], I32, name="gidx")
    Gt = wpool.tile([128, NG * 1024], F16, name="gather")
    fm2v = fm2.rearrange("b y x c -> (b y x) c")            # [32768, 512] f16

    # single full-width chain, all on vector: its ops are ~1.6x faster than
    # gpsimd's and cross-engine joins cost more than they save
    V.tensor_scalar(out=xs[:], in0=dxc, scalar1=cx_ap, scalar2=None, op0=OP.mult)
    V.tensor_tensor(out=xs[:], in0=xs[:], in1=x1c, op=OP.add)
    V.tensor_scalar(out=x0f[:], in0=xs[:], scalar1=-0.5, scalar2=MAGIC,
                    op0=OP.add, op1=OP.add)
    V.tensor_scalar(out=x0f[:], in0=x0f[:], scalar1=-MAGIC, scalar2=None, op0=OP.add)
    V.tensor_scalar(out=ys[:], in0=dyc, scalar1=cy_ap, scalar2=None, op0=OP.mult)
    V.tensor_tensor(out=ys[:], in0=ys[:], in1=y1c, op=OP.add)
    # y0 = round(ys-0.5) via fp32 magic; consistent-pair bilinear stays exact
    V.tensor_scalar(out=y0f[:], in0=ys[:], scalar1=-0.5, scalar2=MAGIC,
                    op0=OP.add, op1=OP.add)
    V.tensor_scalar(out=y0f[:], in0=y0f[:], scalar1=-MAGIC, scalar2=None, op0=OP.add)
    # pix = y0*W + x0 (image offset pre-folded into y coords on host)
    V.tensor_scalar(out=pixf[:], in0=y0f[:], scalar1=float(W), scalar2=None, op0=OP.mult)
    V.tensor_tensor(out=pixf[:], in0=pixf[:], in1=x0f[:], op=OP.add)
    V.tensor_scalar(out=idx[:], in0=pixf[:], scalar1=0.0,
                    scalar2=float(NPIX - 2), op0=OP.max, op1=OP.min)

    # ---------------- gather: 6 indirect DMAs, 2KB descriptors ----------
    for g in range(NG):
        GP.indirect_dma_start(
            out=Gt[:, g * 1024:(g + 1) * 1024],
            out_offset=None,
            in_=fm2v,
            in_offset=bass.IndirectOffsetOnAxis(ap=idx[:, g:g + 1], axis=0),
        )

    # Deferred W1 load in two halves, each gated on real gather data (RAW dep
    # the scheduler cannot hoist): keeps the 1 MB transfer's queue work from
    # delaying the gather transfers. W1P is step-major (s = h*16+q), so the
    # first half covers L1's h=0 matmuls and lands well before they run.
    GP.tensor_copy(out=W1sb[0:1, 0:1], in_=Gt[0:1, 2 * 1024:2 * 1024 + 1])
    nc.sync.dma_start(W1sb[:, 0:2048], w1p[:, 0:2048])
    GP.tensor_copy(out=W1sb[0:1, 2048:2049], in_=Gt[0:1, 4 * 1024:4 * 1024 + 1])
    nc.sync.dma_start(W1sb[:, 2048:4096], w1p[:, 2048:4096])

    # bilinear corner weights (vector; overlaps the gathers)
    ly, lx, hy, hx = t6("ly"), t6("lx"), t6("hy"), t6("hx")
    V.tensor_tensor(out=ly[:], in0=ys[:], in1=y0f[:], op=OP.subtract)
    V.tensor_tensor(out=lx[:], in0=xs[:], in1=x0f[:], op=OP.subtract)
    V.tensor_scalar(out=hy[:], in0=ly[:], scalar1=-1.0, scalar2=1.0, op0=OP.mult, op1=OP.add)
    V.tensor_scalar(out=hx[:], in0=lx[:], scalar1=-1.0, scalar2=1.0, op0=OP.mult, op1=OP.add)
    wc = cpool.tile([128, 24], F16, name="wcat")   # free = (g, xx, yy)
    wv = wc[:, :].rearrange("p (g x y) -> p g x y", g=NG, x=2)
    V.tensor_tensor(out=wv[:, :, 0, 0], in0=hx[:], in1=hy[:], op=OP.mult)
    V.tensor_tensor(out=wv[:, :, 0, 1], in0=hx[:], in1=ly[:], op=OP.mult)
    V.tensor_tensor(out=wv[:, :, 1, 0], in0=lx[:], in1=hy[:], op=OP.mult)
    V.tensor_tensor(out=wv[:, :, 1, 1], in0=lx[:], in1=ly[:], op=OP.mult)
    # ones row for the L3 bias fold (l2ext row 64 = 1.0)
    l2ext = wpool.tile([HID2 + 1, NRS], F16, name="l2ext")
    V.memset(l2ext[HID2:HID2 + 1, :], 1.0)

    def wc_bcast(g):
        return wc[:, :].rearrange("p (g x y) -> p g x y", g=NG, x=2)[:, g, :, :] \
            .unsqueeze(3).to_broadcast([128, 2, 2, C])

    # expand weights to full c-width on the (idle) scalar engine so the
    # per-group mult reads contiguous operands at full DVE rate
    wfull = wpool.tile([128, NG * 1024], F16, name="wfull")
    for g in range(NG):
        wfg = wfull[:, g * 1024:(g + 1) * 1024].rearrange("p (x y c) -> p x y c", x=2, y=2)
        nc.scalar.copy(out=wfg, in_=wc_bcast(g))

    # ---------------- per-group combine + transpose ----------------------
    sv2 = wpool.tile([128, NG * 512], F16, name="sv2")
    sv = wpool.tile([128, NG * 256], F16, name="sv")
    svT = [wpool.tile([128, NG * 128], F16, name=f"svT{h}") for h in range(2)]
    for g in range(NG):
        Gg = Gt[:, g * 1024:(g + 1) * 1024].rearrange("p (x y c) -> p x y c", x=2, y=2)
        V.tensor_tensor(out=Gg, in0=Gg,
                        in1=wfull[:, g * 1024:(g + 1) * 1024]
                        .rearrange("p (x y c) -> p x y c", x=2, y=2), op=OP.mult)
        s2g = sv2[:, g * 512:(g + 1) * 512].rearrange("p (x c) -> p x c", x=2)
        V.tensor_tensor(out=s2g, in0=Gg[:, :, 0, :], in1=Gg[:, :, 1, :], op=OP.add)
        V.tensor_tensor(out=sv[:, g * 256:(g + 1) * 256],
                        in0=s2g[:, 0, :], in1=s2g[:, 1, :], op=OP.add)
        for h in range(2):
            pt = ppool.tile([128, 128], F16, tag="pt", bufs=4, name="pt")
            nc.tensor.transpose(out=pt[:],
                                in_=sv[:, g * 256 + h * 128: g * 256 + (h + 1) * 128],
                                identity=ident)
            nc.scalar.copy(out=svT[h][:, g * 128:(g + 1) * 128], in_=pt[:])

    # ---------------- MLP ----------------
    # psum1 columns j = a*6 + b = roi slot (a = A in 0..7, b = g in 0..5)
    psum1 = ppool.tile([128, NRS], F32, name="psum1")
    for h in range(2):
        for q in range(16):
            s = h * 16 + q
            rhs = svT[h][:, :].rearrange("p (b a s) -> p a b s", b=6, a=8)[:, :, :, q]
            nc.tensor.matmul(out=psum1[:], lhsT=W1sb[:, s * 128:(s + 1) * 128], rhs=rhs,
                             start=(h == 0 and q == 0), stop=(h == 1 and q == 15))
    l1 = wpool.tile([128, NRS], F16, name="l1")
    nc.scalar.activation(out=l1[:], in_=psum1[:], func=AF.Relu, bias=b1sb[:, 0:1], scale=1.0)

    psum2 = ppool.tile([HID2, NRS], F32, name="psum2")
    nc.tensor.matmul(out=psum2[:], lhsT=W2sb, rhs=l1[:], start=True, stop=True)
    nc.scalar.activation(out=l2ext[0:HID2, :], in_=psum2[:], func=AF.Relu,
                         bias=b2sb[:, 0:1], scale=1.0)

    # L3 with b3 folded in: l2ext has a ones row, W3sb row 64 holds b3
    psum3 = ppool.tile([NRS, NUM_CLASSES], F32, name="psum3")
    nc.tensor.matmul(out=psum3[:], lhsT=l2ext[:], rhs=W3sb, start=True, stop=True)

    # ---------------- softmax (rows 0..43 only, fp32) ----------------
    # logits are tiny here (|x| < ~5), so fp32 exp needs no max-subtraction
    ex = wpool.tile([NROI, NUM_CLASSES], F32, name="ex")
    nc.scalar.activation(out=ex[:], in_=psum3[0:NROI, :], func=AF.Exp, bias=0.0, scale=1.0)
    ssum = wpool.tile([NROI, 1], F32, name="ssum")
    V.tensor_reduce(out=ssum[:], in_=ex[:], axis=AX_X, op=OP.add)
    rinv = wpool.tile([NROI, 1], F32, name="rinv")
    V.reciprocal(rinv[:], ssum[:])
    probs = wpool.tile([NROI, NUM_CLASSES], F32, name="probs")
    V.tensor_scalar(out=probs[:], in0=ex[:], scalar1=rinv[:, 0:1], scalar2=None, op0=OP.mult)

    nc.sync.dma_start(out.rearrange("b p c -> (b p) c"), probs[:])


def emit_kernel(nc, tc, fm2, cb, fb, w1p, out):
    with (
        tc.tile_pool(name="const", bufs=1) as cpool,
        tc.tile_pool(name="work", bufs=1) as wpool,
        tc.tile_pool(name="psum", bufs=1, space="PSUM") as ppool,
    ):
        _emit_body(nc, tc, fm2, cb, fb, w1p, out, cpool, wpool, ppool)


def build_module():
    nc = bacc.Bacc(get_trn_type() or "TRN2", target_bir_lowering=False, debug=False)
    fm2 = nc.dram_tensor("fm2", [B_LOC, H, W, 2 * C], F16, kind="ExternalInput")
    cb = nc.dram_tensor("cb", [128, NCB], F32, kind="ExternalInput")
    fb = nc.dram_tensor("fb", [128, NFB], F16, kind="ExternalInput")
    w1p = nc.dram_tensor("w1p", [128, 4096], F16, kind="ExternalInput")
    out = nc.dram_tensor("out", [B_LOC, P, NUM_CLASSES], F32, kind="ExternalOutput")

    with tile.TileContext(nc) as tc:
        emit_kernel(nc, tc, fm2[:], cb[:], fb[:], w1p[:], out[:])
    nc.compile()
    return nc


_NC_CACHE = None


def _get_module():
    global _NC_CACHE
    if _NC_CACHE is None:
        _NC_CACHE = build_module()
    return _NC_CACHE


def _host_prep(inputs):
    """Layout/dtype-only host prep: shard + reformat inputs for the 8 cores."""
    fm = np.ascontiguousarray(np.asarray(inputs["feature_map"], np.float32))
    prop = np.asarray(inputs["proposals"], np.float32)
    W1 = np.asarray(inputs["W1"], np.float32)
    b1 = np.asarray(inputs["b1"], np.float32)
    W2 = np.asarray(inputs["W2"], np.float32)
    b2 = np.asarray(inputs["b2"], np.float32)
    W3 = np.asarray(inputs["W3"], np.float32)
    b3 = np.asarray(inputs["b3"], np.float32)

    # y-pair-interleaved fp16 feature map: fm2[b,y,x] = [fm[y,x], fm[y+1,x]]
    fm16 = fm.astype(np.float16)
    fm2 = np.empty((B, H, W, 2 * C), np.float16)
    fm2[:, :, :, :C] = fm16
    fm2[:, :H - 1, :, C:] = fm16[:, 1:]
    fm2[:, H - 1, :, C:] = fm16[:, H - 1]

    # fp16 const bundle: ident | W2 | W3 (shared by all cores)
    fb = np.zeros((128, NFB), np.float16)
    fb[:, 0:128] = np.eye(128, dtype=np.float16)
    fb[:, 128:128 + HID2] = W2.astype(np.float16)
    fb[0:HID2, 128 + HID2:NFB] = W3.astype(np.float16)
    fb[HID2, 128 + HID2:NFB] = b3.astype(np.float16)

    # W1 in step-major SBUF lhsT layout: for L1 step s = h*16+q,
    # W1P[p, s*128 + m] = W1[q*256 + h*128 + p, m]
    w1p = np.ascontiguousarray(
        W1.reshape(16, 2, 128, HID1).transpose(2, 1, 0, 3).reshape(128, 4096)
    ).astype(np.float16)

    # per-slot roi ids: slot (A, g) -> roi min(A*6+g, 43)
    roi_of_slot = np.minimum(np.arange(8)[:, None] * 6 + np.arange(6)[None, :], NROI - 1)
    q = np.arange(128) % 16
    cy = ((q // 4).astype(np.float32) + 0.5) / 4.0
    cx = ((q % 4).astype(np.float32) + 0.5) / 4.0

    in_maps = []
    for c in range(N_CORES):
        sl = slice(B_LOC * c, B_LOC * (c + 1))
        pv = prop[sl].reshape(NROI, 4).copy()
        pv[P:, 0] += H   # fold image-1 offset into y coords
        pv[P:, 2] += H
        # pass (y1, x1, dy, dx) per slot — the kernel chain needs the deltas
        pv[:, 2] -= pv[:, 0]
        pv[:, 3] -= pv[:, 1]
        cb = np.zeros((128, NCB), np.float32)
        cb[:, 0:24] = np.repeat(pv[roi_of_slot].reshape(8, 24), 16, axis=0)
        cb[:, 24] = cy
        cb[:, 25] = cx
        cb[:, 26] = b1
        cb[0:HID2, 27] = b2
        cb[:, 28:38] = b3[None, :]
        in_maps.append({
            "fm2": fm2[sl],
            "cb": cb,
            "fb": fb,
            "w1p": w1p,
        })
    return in_maps


def run(inputs, trace=False):
    """Run on all 8 cores; returns (output [16,22,10], BassKernelResults)."""
    nc = _get_module()
    res = run_bass_kernel_spmd(nc, _host_prep(inputs), core_ids=list(range(N_CORES)),
                               trace=trace)
    out = np.concatenate([r["out"] for r in res.results], axis=0)
    return out, res


def kernel(**inputs) -> np.ndarray:
    out, _ = run(inputs, trace=False)
    return out
